# revision 21
# baseline (speedup 1.0000x reference)
"""Trainium2 Bass kernel for the soft-logic-gate CA problem.

Math (per sample, grid 128x128, 4 layers):
  state' = clip( sum_m sigmoid(tg[l,m]) * prod_j g(bit_j(m), tap_j), 0, 1 )
  taps: A=state[x,y], B=state[x,y+1], C=state[x+1,y], D=state[x+1,y+1] (periodic)
  g(0,t)=1-t, g(1,t)=t;  m = bA*8 + bB*4 + bC*2 + bD.

4-D multilinear interpolation of the 16 gate maps at corner (A,B,C,D).
The sigmoided gates are converted OFFLINE (host numpy, input-independent
weight preprocessing) to multilinear-polynomial coefficients via the
Moebius transform (c[m] -= c[m-bit]); the device evaluates each layer
with a Horner butterfly of fp16 tensor_tensor ops (A, then B, C, D).

Layout: partition = grid row (128).  State is parity planes (b, t, k):
t=0 even grid cols, t=1 odd.  The periodic column shift y+1 (B/D taps)
is materialized once per layer by the ACT engine into contiguous tap
tiles (stB from state, srB from the rowshift PSUM result), so every DVE
Horner op is a single large 2x-mode tensor_tensor with no wrap-column
splits.  Row shifts (x+1): layer 0 reads host-prepped rolled copies of
x; layers 1-3 use a PE permutation matmul + ACT copy-back.  All
coefficients arrive fp16 in final (m, t, k) layouts: no on-chip
sigmoid, Moebius, or casts.  Layer 0 ships only its 8 live coefficients
(B=D=0 initially), layer 3 only even-column ones.

Sharding: batch 32 -> 8 cores x 4 samples (coefficients replicated).
Engines: DVE does all Horner + clamps (GpSimd stays idle: its SBUF port
is shared with the DVE 2x read port, so concurrent Pool work stalls
DVE); ACT builds tap tiles + PSUM copy-backs; PE the row-shift matmuls;
DMA on the two HW-DGE queues (sync, scalar).
"""

import numpy as np

import concourse.bacc as bacc
import concourse.mybir as mybir
from concourse.ap import AP
from concourse.tile import TileContext
from concourse.bass_utils import run_bass_kernel_spmd

F32 = mybir.dt.float32
DT = mybir.dt.float16  # compute dtype
AL = mybir.AluOpType
P = 128          # partitions = grid rows
B = 4            # samples per core
Y = 128          # grid cols
K = 64           # x cols (even grid cols)
L = 4
M = 16
N_CORES = 8

SK = 2 * K       # state elems per sample (E|O planes)
CHA = 512 + B * 4 * K   # [g0 mult coeffs | g0 add coeffs | XP4 (b: X Xc X Xc)]
CHB = B * 2 * K         # [XR (b: Xr Xrc)]
O_L2, O_L3 = M * Y, 2 * M * Y
CW = 2 * M * Y + M * K


def _emit(tc, nc, ca_ap, cb_ap, ps_ap, g1_ap, g2_ap, g3_ap, out_ap):
    vec, act = nc.vector, nc.scalar

    def mk(t, off, dims):
        a = t if isinstance(t, AP) else t[:]
        return AP(a.tensor, a.offset + off, [list(a.ap[0])] + dims)

    def tt_(eng, out, in0, in1, op):
        eng.tensor_tensor(out=out, in0=in0, in1=in1, op=op)

    def clamp(out_ap_, in_ap_):
        vec.tensor_scalar(
            out=out_ap_, in0=in_ap_, scalar1=0.0, scalar2=1.0, op0=AL.max, op1=AL.min
        )

    with (
        tc.tile_pool(name="coef", bufs=1) as pc,
        tc.tile_pool(name="st", bufs=2) as pst,
        tc.tile_pool(name="sb", bufs=2) as psb,
        tc.tile_pool(name="sr", bufs=2) as psr,
        tc.tile_pool(name="wk", bufs=1) as pwk,
        tc.tile_pool(name="ps", bufs=2, space="PSUM") as pps,
    ):
        # ---- input DMAs, split across the two HW-DGE queues (FIFO per
        # queue: front-load what layer 0 needs).
        tw = pc.tile([P, CW], DT, tag="tw")
        cha = pwk.tile([P, CHA], DT, tag="cha")
        chb = pwk.tile([P, CHB], DT, tag="chb")
        nc.sync.dma_start(out=cha[:, 0:CHA // 2], in_=ca_ap[:, 0:CHA // 2])
        act.dma_start(out=cha[:, CHA // 2:CHA], in_=ca_ap[:, CHA // 2:CHA])
        act.dma_start(out=chb[:], in_=cb_ap)
        # split g1 across both queues: the A-level mult needs the hi
        # (m=8..15) coefficient block first
        HB = 8 * Y
        nc.sync.dma_start(out=tw[:, HB:2 * HB], in_=g1_ap[:, HB:2 * HB])
        act.dma_start(out=tw[:, 0:HB], in_=g1_ap[:, 0:HB])
        psh = pwk.tile([P, P], DT, tag="psh")
        act.dma_start(out=psh[:], in_=ps_ap)
        nc.sync.dma_start(out=tw[:, O_L2:O_L2 + M * Y], in_=g2_ap)
        act.dma_start(out=tw[:, O_L3:O_L3 + M * K], in_=g3_ap)

        # warm the ACT table bank early so the first copy isn't stuck
        # behind a table load
        scr = pwk.tile([P, 2], F32, tag="scr")
        vec.memset(scr[:], 0.0)
        act.copy(out=scr[:, 1:2], in_=scr[:, 0:1])

        # ---- layer 0 eval: one fused 2-D interp over both parities ----
        # state layout (b, t, k): b*128 + t*64 + k
        # ue layout (b, s2, h2, k): both parity halves in one op set:
        #   h=0 even: s = (c0e + cA*X) + Xr*(cCe + cAC*X)   -> E plane
        #   h=1 odd:  s = (c0o + cB*Xc) + Xrc*(cDo + cBD*Xc) -> O plane
        # cha = [ [cA_e cB_o cAC_e cBD_o] | XP4 (b: X Xc X Xc) ]
        # chb = [ [c0_e c0_o cC_e cD_o]   | XR  (b: Xr Xrc) ]
        st1 = pst.tile([P, B * SK], DT, tag="state")
        ue = pwk.tile([P, 4 * B * K], DT, tag="ue")   # (b, s, h, k)
        te = pwk.tile([P, 2 * B * K], DT, tag="te")   # (b, h, k)

        tt_(vec, mk(ue, 0, [[256, B], [64, 4], [1, K]]),
            mk(cha, 0, [[0, B], [64, 4], [1, K]]),
            mk(cha, 512, [[256, B], [64, 4], [1, K]]), AL.mult)
        tt_(vec, mk(ue, 0, [[256, B], [64, 4], [1, K]]),
            mk(ue, 0, [[256, B], [64, 4], [1, K]]),
            mk(cha, 256, [[0, B], [64, 4], [1, K]]), AL.add)
        tt_(vec, mk(te, 0, [[128, B], [64, 2], [1, K]]),
            mk(ue, 128, [[256, B], [64, 2], [1, K]]),
            mk(chb, 0, [[128, B], [64, 2], [1, K]]), AL.mult)
        tt_(vec, mk(te, 0, [[128, B], [64, 2], [1, K]]),
            mk(te, 0, [[128, B], [64, 2], [1, K]]),
            mk(ue, 0, [[256, B], [64, 2], [1, K]]), AL.add)
        clamp(mk(st1, 0, [[128, B], [1, 128]]), mk(te, 0, [[128, B], [1, 128]]))

        # ---- column-shift tap builder (ACT): dst(b,t,k) = y+1 taps ----
        def colshift(src, src_psum=False):
            dst = psb.tile([P, B * SK], DT, tag="stB")
            act.copy(out=mk(dst, 0, [[128, B], [1, K]]),
                     in_=mk(src, 64, [[128, B], [1, K]]))
            act.copy(out=mk(dst, 64, [[128, B], [1, K - 1]]),
                     in_=mk(src, 1, [[128, B], [1, K - 1]]))
            act.copy(out=mk(dst, 127, [[128, B], [1, 1]]),
                     in_=mk(src, 0, [[128, B], [1, 1]]))
            return dst

        # ---- generic layer eval (A, then B, C, D) ---------------------
        u = pwk.tile([P, 8 * B * SK], DT, tag="u")    # (b, i8, t, k)
        v_t = pwk.tile([P, 4 * B * SK], DT, tag="v")  # (b, j4, t, k)
        w2 = pwk.tile([P, 2 * B * SK], DT, tag="w2")  # (b, j2, t, k)
        tt2 = pwk.tile([P, B * SK], DT, tag="tt")     # (b, t, k)

        def eval_layer12(cofs, st, stB, sr, srB, stn):
            # A level: u_i = cLO_i + cHI_i * A
            tt_(vec, mk(u, 0, [[1024, B], [128, 8], [1, 128]]),
                mk(tw, cofs + 8 * Y, [[0, B], [128, 8], [1, 128]]),
                mk(st, 0, [[128, B], [0, 8], [1, 128]]), AL.mult)
            tt_(vec, mk(u, 0, [[1024, B], [128, 8], [1, 128]]),
                mk(u, 0, [[1024, B], [128, 8], [1, 128]]),
                mk(tw, cofs, [[0, B], [128, 8], [1, 128]]), AL.add)
            # B level: v_j = u_j + u_{4+j} * Btap
            tt_(vec, mk(v_t, 0, [[512, B], [128, 4], [1, 128]]),
                mk(u, 512, [[1024, B], [128, 4], [1, 128]]),
                mk(stB, 0, [[128, B], [0, 4], [1, 128]]), AL.mult)
            tt_(vec, mk(v_t, 0, [[512, B], [128, 4], [1, 128]]),
                mk(v_t, 0, [[512, B], [128, 4], [1, 128]]),
                mk(u, 0, [[1024, B], [128, 4], [1, 128]]), AL.add)
            # C level: w_j = v_j + v_{2+j} * C          (C = sr planes)
            tt_(vec, mk(w2, 0, [[256, B], [128, 2], [1, 128]]),
                mk(v_t, 256, [[512, B], [128, 2], [1, 128]]),
                mk(sr, 0, [[128, B], [0, 2], [1, 128]]), AL.mult)
            tt_(vec, mk(w2, 0, [[256, B], [128, 2], [1, 128]]),
                mk(w2, 0, [[256, B], [128, 2], [1, 128]]),
                mk(v_t, 0, [[512, B], [128, 2], [1, 128]]), AL.add)
            # D level: s = w_0 + w_1 * Dtap
            tt_(vec, mk(tt2, 0, [[128, B], [1, 128]]),
                mk(w2, 128, [[256, B], [1, 128]]),
                mk(srB, 0, [[128, B], [1, 128]]), AL.mult)
            tt_(vec, mk(tt2, 0, [[128, B], [1, 128]]),
                mk(tt2, 0, [[128, B], [1, 128]]),
                mk(w2, 0, [[256, B], [1, 128]]), AL.add)
            clamp(stn[:], tt2[:])

        st = st1
        for l in (1, 2):
            cofs = 0 if l == 1 else O_L2
            # PE rowshift matmul first (independent engine), then ACT tap
            # builds ordered by when DVE consumes them: stB (B level),
            # sr (C level), srB (D level)
            pt = pps.tile([P, B * SK], F32, tag="psum")
            nc.tensor.matmul(pt[:], psh[:], st[:], start=True, stop=True)
            stB = colshift(st)
            sr = psr.tile([P, B * SK], DT, tag="sr")
            act.copy(out=sr[:], in_=pt[:])
            srB = colshift(pt)
            stn = pst.tile([P, B * SK], DT, tag="state")
            eval_layer12(cofs, st, stB, sr, srB, stn)
            st = stn

        # ---- layer 3 (even outputs only, plane taps, no wraps) --------
        pt3 = pps.tile([P, B * SK], F32, tag="psum")
        nc.tensor.matmul(pt3[:], psh[:], st[:], start=True, stop=True)
        sr3 = psr.tile([P, B * SK], DT, tag="sr")
        act.copy(out=sr3[:], in_=pt3[:])
        tt_(vec, mk(u, 0, [[512, B], [64, 8], [1, K]]),
            mk(tw, O_L3 + 8 * K, [[0, B], [64, 8], [1, K]]),
            mk(st, 0, [[128, B], [0, 8], [1, K]]), AL.mult)
        tt_(vec, mk(u, 0, [[512, B], [64, 8], [1, K]]),
            mk(u, 0, [[512, B], [64, 8], [1, K]]),
            mk(tw, O_L3, [[0, B], [64, 8], [1, K]]), AL.add)
        tt_(vec, mk(v_t, 0, [[256, B], [64, 4], [1, K]]),
            mk(u, 256, [[512, B], [64, 4], [1, K]]),
            mk(st, 64, [[128, B], [0, 4], [1, K]]), AL.mult)
        tt_(vec, mk(v_t, 0, [[256, B], [64, 4], [1, K]]),
            mk(v_t, 0, [[256, B], [64, 4], [1, K]]),
            mk(u, 0, [[512, B], [64, 4], [1, K]]), AL.add)
        tt_(vec, mk(w2, 0, [[128, B], [64, 2], [1, K]]),
            mk(v_t, 128, [[256, B], [64, 2], [1, K]]),
            mk(sr3, 0, [[128, B], [0, 2], [1, K]]), AL.mult)
        tt_(vec, mk(w2, 0, [[128, B], [64, 2], [1, K]]),
            mk(w2, 0, [[128, B], [64, 2], [1, K]]),
            mk(v_t, 0, [[256, B], [64, 2], [1, K]]), AL.add)
        # D level + output: split by b-halves, alternate stores across
        # both HW-DGE queues so the tail store is never queued behind one
        out_t = pwk.tile([P, B * K], DT, tag="out")
        for h in (0, 1):
            o = h * 128          # tt2/out_t half offset (b-stride 64)
            q = h * 256          # w2/sr3 half offset (b-stride 128)
            tt_(vec, mk(tt2, o, [[64, 2], [1, K]]),
                mk(w2, 64 + q, [[128, 2], [1, K]]),
                mk(sr3, 64 + q, [[128, 2], [1, K]]), AL.mult)
            tt_(vec, mk(tt2, o, [[64, 2], [1, K]]),
                mk(tt2, o, [[64, 2], [1, K]]),
                mk(w2, q, [[128, 2], [1, K]]), AL.add)
            clamp(mk(out_t, o, [[64, 1], [1, K]]), mk(tt2, o, [[64, 1], [1, K]]))
            (nc.sync if h == 0 else act).dma_start(
                out=out_ap[:, o:o + K], in_=out_t[:, o:o + K])
            clamp(mk(out_t, o + K, [[64, 1], [1, K]]),
                  mk(tt2, o + K, [[64, 1], [1, K]]))
            (act if h == 0 else nc.sync).dma_start(
                out=out_ap[:, o + K:o + 128], in_=out_t[:, o + K:o + 128])


_NC_CACHE = {}


def build():
    if "nc" in _NC_CACHE:
        return _NC_CACHE["nc"]
    nc = bacc.Bacc(
        "TRN2",
        target_bir_lowering=False,
        debug=False,
        enable_asserts=False,
        num_devices=N_CORES,
    )
    ca_d = nc.dram_tensor("cha", (P, CHA), DT, kind="ExternalInput")
    cb_d = nc.dram_tensor("chb", (P, CHB), DT, kind="ExternalInput")
    ps_d = nc.dram_tensor("pshift", (P, P), DT, kind="ExternalInput")
    g1_d = nc.dram_tensor("g1", (P, M * Y), DT, kind="ExternalInput")
    g2_d = nc.dram_tensor("g2", (P, M * Y), DT, kind="ExternalInput")
    g3_d = nc.dram_tensor("g3", (P, M * K), DT, kind="ExternalInput")
    out_d = nc.dram_tensor("out", (P, B * K), DT, kind="ExternalOutput")
    with TileContext(nc) as tc:
        _emit(tc, nc, ca_d.ap(), cb_d.ap(), ps_d.ap(), g1_d.ap(), g2_d.ap(),
              g3_d.ap(), out_d.ap())
    nc.compile()
    _NC_CACHE["nc"] = nc
    return nc


def _moebius_coeffs(toggle_gates):
    """sigmoid + Moebius transform of the gate maps -> multilinear coeffs.

    Input-independent weight preprocessing (exact math); returns
    (L, 16, d1, d2) float32 with m = bA*8 + bB*4 + bC*2 + bD.
    """
    tg = np.asarray(toggle_gates, dtype=np.float64)
    c = 1.0 / (1.0 + np.exp(-tg))                       # sigmoid
    c = c.reshape(L, 2, 2, 2, 2, P, Y)                  # (l, bA, bB, bC, bD, x, y)
    for ax in (1, 2, 3, 4):
        hi = [slice(None)] * 7
        lo = [slice(None)] * 7
        hi[ax] = 1
        lo[ax] = 0
        c[tuple(hi)] -= c[tuple(lo)]
    return c.reshape(L, M, P, Y).astype(np.float32)


def make_in_maps(x, toggle_gates):
    x = np.asarray(x, dtype=np.float32)
    c = _moebius_coeffs(toggle_gates)
    # layer 0: only S within {A,C} (even outputs) / {B,D} (odd) survive.
    # mult coeffs (s,h): [cA_e, cB_o, cAC_e, cBD_o]; add: [c0e, c0o, cCe, cDo]
    ev, od = c[0][:, :, 0::2], c[0][:, :, 1::2]
    g0m = np.stack([ev[8], od[4], ev[10], od[5]])
    g0a = np.stack([ev[0], od[0], ev[2], od[1]])
    g0m = g0m.transpose(1, 0, 2).reshape(P, 4 * K)
    g0a = g0a.transpose(1, 0, 2).reshape(P, 4 * K)

    def gl(l):
        a = c[l].transpose(1, 0, 2).reshape(P, M, K, 2)    # (P, m, k, t)
        return np.ascontiguousarray(
            a.transpose(0, 1, 3, 2).reshape(P, M * Y), dtype=np.float16)

    g1, g2 = gl(1), gl(2)
    g3 = np.ascontiguousarray(
        c[3][:, :, 0::2].transpose(1, 0, 2).reshape(P, M * K), dtype=np.float16)
    psm = np.eye(P, k=-1, dtype=np.float64)
    psm[0, P - 1] = 1.0
    psm = psm.astype(np.float16)
    xr = np.roll(x, -1, axis=1)                            # row shift (x+1)
    ins = []
    for cc in range(N_CORES):
        xs = x[cc * B:(cc + 1) * B]                        # (B, P, K)
        xf = xs.transpose(1, 0, 2).reshape(P, B, K)
        xrf = xr[cc * B:(cc + 1) * B].transpose(1, 0, 2).reshape(P, B, K)
        xcf = np.roll(xf, -1, axis=2)
        xrcf = np.roll(xrf, -1, axis=2)
        xp4 = np.stack([xf, xcf, xf, xcf], axis=2).reshape(P, B * 4 * K)
        xr2 = np.stack([xrf, xrcf], axis=2).reshape(P, B * 2 * K)
        cha = np.ascontiguousarray(
            np.concatenate([g0m, g0a, xp4], axis=1), dtype=np.float16)
        chb = np.ascontiguousarray(xr2, dtype=np.float16)
        ins.append({"cha": cha, "chb": chb, "pshift": psm,
                    "g1": g1, "g2": g2, "g3": g3})
    return ins


def kernel(x, toggle_gates):
    nc = build()
    res = run_bass_kernel_spmd(
        nc, make_in_maps(x, toggle_gates), core_ids=list(range(N_CORES))
    )
    outs = []
    for cc in range(N_CORES):
        o = res.results[cc]["out"].reshape(P, B, K).transpose(1, 0, 2)
        outs.append(o)
    return np.ascontiguousarray(np.concatenate(outs, axis=0), dtype=np.float32)


# revision 22
# speedup vs baseline: 1.0237x; 1.0237x over previous
"""Trainium2 Bass kernel for the soft-logic-gate CA problem.

Math (per sample, grid 128x128, 4 layers):
  state' = clip( sum_m sigmoid(tg[l,m]) * prod_j g(bit_j(m), tap_j), 0, 1 )
  taps: A=state[x,y], B=state[x,y+1], C=state[x+1,y], D=state[x+1,y+1] (periodic)
  g(0,t)=1-t, g(1,t)=t;  m = bA*8 + bB*4 + bC*2 + bD.

4-D multilinear interpolation of the 16 gate maps at corner (A,B,C,D).
The sigmoided gates are converted OFFLINE (host numpy, input-independent
weight preprocessing) to multilinear-polynomial coefficients via the
Moebius transform (c[m] -= c[m-bit]); the device evaluates each layer
with a Horner butterfly of fp16 tensor_tensor ops (A, then B, C, D).

Layout: partition = grid row (128).  State is parity planes (b, t, k):
t=0 even grid cols, t=1 odd.  The periodic column shift y+1 (B/D taps)
is materialized once per layer by the ACT engine into contiguous tap
tiles (stB from state, srB from the rowshift PSUM result), so every DVE
Horner op is a single large 2x-mode tensor_tensor with no wrap-column
splits.  Row shifts (x+1): layer 0 reads host-prepped rolled copies of
x; layers 1-3 use a PE permutation matmul + ACT copy-back.  All
coefficients arrive fp16 in final (m, t, k) layouts: no on-chip
sigmoid, Moebius, or casts.  Layer 0 ships only its 8 live coefficients
(B=D=0 initially), layer 3 only even-column ones.

Sharding: batch 32 -> 8 cores x 4 samples (coefficients replicated).
Engines: DVE does all Horner + clamps (GpSimd stays idle: its SBUF port
is shared with the DVE 2x read port, so concurrent Pool work stalls
DVE); ACT builds tap tiles + PSUM copy-backs; PE the row-shift matmuls;
DMA on the two HW-DGE queues (sync, scalar).
"""

import numpy as np

import concourse.bacc as bacc
import concourse.mybir as mybir
from concourse.ap import AP
from concourse.tile import TileContext
from concourse.bass_utils import run_bass_kernel_spmd

F32 = mybir.dt.float32
DT = mybir.dt.float16  # compute dtype
AL = mybir.AluOpType
P = 128          # partitions = grid rows
B = 4            # samples per core
Y = 128          # grid cols
K = 64           # x cols (even grid cols)
L = 4
M = 16
N_CORES = 8

SK = 2 * K       # state elems per sample (E|O planes)
CHA = 512 + B * 4 * K   # [g0 mult coeffs | g0 add coeffs | XP4 (b: X Xc X Xc)]
CHB = B * 2 * K         # [XR (b: Xr Xrc)]
O_L2, O_L3 = M * Y, 2 * M * Y
CW = 2 * M * Y + M * K


def _emit(tc, nc, ca_ap, cb_ap, ps_ap, g1_ap, g2_ap, g3_ap, out_ap):
    vec, act = nc.vector, nc.scalar

    def mk(t, off, dims):
        a = t if isinstance(t, AP) else t[:]
        return AP(a.tensor, a.offset + off, [list(a.ap[0])] + dims)

    def tt_(eng, out, in0, in1, op):
        eng.tensor_tensor(out=out, in0=in0, in1=in1, op=op)

    def clamp(out_ap_, in_ap_):
        vec.tensor_scalar(
            out=out_ap_, in0=in_ap_, scalar1=0.0, scalar2=1.0, op0=AL.max, op1=AL.min
        )

    with (
        tc.tile_pool(name="coef", bufs=1) as pc,
        tc.tile_pool(name="st", bufs=2) as pst,
        tc.tile_pool(name="sb", bufs=2) as psb,
        tc.tile_pool(name="sr", bufs=2) as psr,
        tc.tile_pool(name="wk", bufs=1) as pwk,
        tc.tile_pool(name="ps", bufs=2, space="PSUM") as pps,
    ):
        # ---- input DMAs, split across the two HW-DGE queues (FIFO per
        # queue: front-load what layer 0 needs).
        tw = pc.tile([P, CW], DT, tag="tw")
        cha = pwk.tile([P, CHA], DT, tag="cha")
        chb = pwk.tile([P, CHB], DT, tag="chb")
        nc.sync.dma_start(out=cha[:], in_=ca_ap)
        act.dma_start(out=chb[:], in_=cb_ap)
        # split g1 across both queues: the A-level mult needs the hi
        # (m=8..15) coefficient block first
        HB = 8 * Y
        nc.sync.dma_start(out=tw[:, HB:2 * HB], in_=g1_ap[:, HB:2 * HB])
        act.dma_start(out=tw[:, 0:HB], in_=g1_ap[:, 0:HB])
        psh = pwk.tile([P, P], DT, tag="psh")
        act.dma_start(out=psh[:], in_=ps_ap)
        nc.sync.dma_start(out=tw[:, O_L2:O_L2 + M * Y], in_=g2_ap)
        act.dma_start(out=tw[:, O_L3:O_L3 + M * K], in_=g3_ap)

        # warm the ACT table bank early so the first copy isn't stuck
        # behind a table load
        scr = pwk.tile([P, 2], F32, tag="scr")
        vec.memset(scr[:], 0.0)
        act.copy(out=scr[:, 1:2], in_=scr[:, 0:1])

        # ---- layer 0 eval: one fused 2-D interp over both parities ----
        # state layout (b, t, k): b*128 + t*64 + k
        # ue layout (b, s2, h2, k): both parity halves in one op set:
        #   h=0 even: s = (c0e + cA*X) + Xr*(cCe + cAC*X)   -> E plane
        #   h=1 odd:  s = (c0o + cB*Xc) + Xrc*(cDo + cBD*Xc) -> O plane
        # cha = [ [cA_e cB_o cAC_e cBD_o] | XP4 (b: X Xc X Xc) ]
        # chb = [ [c0_e c0_o cC_e cD_o]   | XR  (b: Xr Xrc) ]
        st1 = pst.tile([P, B * SK], DT, tag="state")
        ue = pwk.tile([P, 4 * B * K], DT, tag="ue")   # (b, s, h, k)
        te = pwk.tile([P, 2 * B * K], DT, tag="te")   # (b, h, k)

        tt_(vec, mk(ue, 0, [[256, B], [64, 4], [1, K]]),
            mk(cha, 0, [[0, B], [64, 4], [1, K]]),
            mk(cha, 512, [[256, B], [64, 4], [1, K]]), AL.mult)
        tt_(vec, mk(ue, 0, [[256, B], [64, 4], [1, K]]),
            mk(ue, 0, [[256, B], [64, 4], [1, K]]),
            mk(cha, 256, [[0, B], [64, 4], [1, K]]), AL.add)
        tt_(vec, mk(te, 0, [[128, B], [64, 2], [1, K]]),
            mk(ue, 128, [[256, B], [64, 2], [1, K]]),
            mk(chb, 0, [[128, B], [64, 2], [1, K]]), AL.mult)
        tt_(vec, mk(te, 0, [[128, B], [64, 2], [1, K]]),
            mk(te, 0, [[128, B], [64, 2], [1, K]]),
            mk(ue, 0, [[256, B], [64, 2], [1, K]]), AL.add)
        clamp(mk(st1, 0, [[128, B], [1, 128]]), mk(te, 0, [[128, B], [1, 128]]))

        # ---- column-shift tap builder (ACT): dst(b,t,k) = y+1 taps ----
        def colshift(src, src_psum=False):
            dst = psb.tile([P, B * SK], DT, tag="stB")
            act.copy(out=mk(dst, 0, [[128, B], [1, K]]),
                     in_=mk(src, 64, [[128, B], [1, K]]))
            act.copy(out=mk(dst, 64, [[128, B], [1, K - 1]]),
                     in_=mk(src, 1, [[128, B], [1, K - 1]]))
            act.copy(out=mk(dst, 127, [[128, B], [1, 1]]),
                     in_=mk(src, 0, [[128, B], [1, 1]]))
            return dst

        # ---- generic layer eval (A, then B, C, D) ---------------------
        u = pwk.tile([P, 8 * B * SK], DT, tag="u")    # (b, i8, t, k)
        v_t = pwk.tile([P, 4 * B * SK], DT, tag="v")  # (b, j4, t, k)
        w2 = pwk.tile([P, 2 * B * SK], DT, tag="w2")  # (b, j2, t, k)
        tt2 = pwk.tile([P, B * SK], DT, tag="tt")     # (b, t, k)

        def eval_layer12(cofs, st, stB, sr, srB, stn):
            # A level: u_i = cLO_i + cHI_i * A
            tt_(vec, mk(u, 0, [[1024, B], [128, 8], [1, 128]]),
                mk(tw, cofs + 8 * Y, [[0, B], [128, 8], [1, 128]]),
                mk(st, 0, [[128, B], [0, 8], [1, 128]]), AL.mult)
            tt_(vec, mk(u, 0, [[1024, B], [128, 8], [1, 128]]),
                mk(u, 0, [[1024, B], [128, 8], [1, 128]]),
                mk(tw, cofs, [[0, B], [128, 8], [1, 128]]), AL.add)
            # B level: v_j = u_j + u_{4+j} * Btap
            tt_(vec, mk(v_t, 0, [[512, B], [128, 4], [1, 128]]),
                mk(u, 512, [[1024, B], [128, 4], [1, 128]]),
                mk(stB, 0, [[128, B], [0, 4], [1, 128]]), AL.mult)
            tt_(vec, mk(v_t, 0, [[512, B], [128, 4], [1, 128]]),
                mk(v_t, 0, [[512, B], [128, 4], [1, 128]]),
                mk(u, 0, [[1024, B], [128, 4], [1, 128]]), AL.add)
            # C level: w_j = v_j + v_{2+j} * C          (C = sr planes)
            tt_(vec, mk(w2, 0, [[256, B], [128, 2], [1, 128]]),
                mk(v_t, 256, [[512, B], [128, 2], [1, 128]]),
                mk(sr, 0, [[128, B], [0, 2], [1, 128]]), AL.mult)
            tt_(vec, mk(w2, 0, [[256, B], [128, 2], [1, 128]]),
                mk(w2, 0, [[256, B], [128, 2], [1, 128]]),
                mk(v_t, 0, [[512, B], [128, 2], [1, 128]]), AL.add)
            # D level: s = w_0 + w_1 * Dtap
            tt_(vec, mk(tt2, 0, [[128, B], [1, 128]]),
                mk(w2, 128, [[256, B], [1, 128]]),
                mk(srB, 0, [[128, B], [1, 128]]), AL.mult)
            tt_(vec, mk(tt2, 0, [[128, B], [1, 128]]),
                mk(tt2, 0, [[128, B], [1, 128]]),
                mk(w2, 0, [[256, B], [1, 128]]), AL.add)
            clamp(stn[:], tt2[:])

        st = st1
        for l in (1, 2):
            cofs = 0 if l == 1 else O_L2
            # PE rowshift matmul first (independent engine), then ACT tap
            # builds ordered by when DVE consumes them: stB (B level),
            # sr (C level), srB (D level)
            pt = pps.tile([P, B * SK], F32, tag="psum")
            nc.tensor.matmul(pt[:], psh[:], st[:], start=True, stop=True)
            stB = colshift(st)
            sr = psr.tile([P, B * SK], DT, tag="sr")
            act.copy(out=sr[:], in_=pt[:])
            srB = colshift(pt)
            stn = pst.tile([P, B * SK], DT, tag="state")
            eval_layer12(cofs, st, stB, sr, srB, stn)
            st = stn

        # ---- layer 3 (even outputs only, plane taps, no wraps) --------
        pt3 = pps.tile([P, B * SK], F32, tag="psum")
        nc.tensor.matmul(pt3[:], psh[:], st[:], start=True, stop=True)
        sr3 = psr.tile([P, B * SK], DT, tag="sr")
        act.copy(out=sr3[:], in_=pt3[:])
        tt_(vec, mk(u, 0, [[512, B], [64, 8], [1, K]]),
            mk(tw, O_L3 + 8 * K, [[0, B], [64, 8], [1, K]]),
            mk(st, 0, [[128, B], [0, 8], [1, K]]), AL.mult)
        tt_(vec, mk(u, 0, [[512, B], [64, 8], [1, K]]),
            mk(u, 0, [[512, B], [64, 8], [1, K]]),
            mk(tw, O_L3, [[0, B], [64, 8], [1, K]]), AL.add)
        tt_(vec, mk(v_t, 0, [[256, B], [64, 4], [1, K]]),
            mk(u, 256, [[512, B], [64, 4], [1, K]]),
            mk(st, 64, [[128, B], [0, 4], [1, K]]), AL.mult)
        tt_(vec, mk(v_t, 0, [[256, B], [64, 4], [1, K]]),
            mk(v_t, 0, [[256, B], [64, 4], [1, K]]),
            mk(u, 0, [[512, B], [64, 4], [1, K]]), AL.add)
        tt_(vec, mk(w2, 0, [[128, B], [64, 2], [1, K]]),
            mk(v_t, 128, [[256, B], [64, 2], [1, K]]),
            mk(sr3, 0, [[128, B], [0, 2], [1, K]]), AL.mult)
        tt_(vec, mk(w2, 0, [[128, B], [64, 2], [1, K]]),
            mk(w2, 0, [[128, B], [64, 2], [1, K]]),
            mk(v_t, 0, [[256, B], [64, 2], [1, K]]), AL.add)
        # D level + output: split by b-halves, alternate stores across
        # both HW-DGE queues so the tail store is never queued behind one
        out_t = pwk.tile([P, B * K], DT, tag="out")
        for h in (0, 1):
            o = h * 128          # tt2/out_t half offset (b-stride 64)
            q = h * 256          # w2/sr3 half offset (b-stride 128)
            tt_(vec, mk(tt2, o, [[64, 2], [1, K]]),
                mk(w2, 64 + q, [[128, 2], [1, K]]),
                mk(sr3, 64 + q, [[128, 2], [1, K]]), AL.mult)
            tt_(vec, mk(tt2, o, [[64, 2], [1, K]]),
                mk(tt2, o, [[64, 2], [1, K]]),
                mk(w2, q, [[128, 2], [1, K]]), AL.add)
            clamp(mk(out_t, o, [[64, 1], [1, K]]), mk(tt2, o, [[64, 1], [1, K]]))
            (nc.sync if h == 0 else act).dma_start(
                out=out_ap[:, o:o + K], in_=out_t[:, o:o + K])
            clamp(mk(out_t, o + K, [[64, 1], [1, K]]),
                  mk(tt2, o + K, [[64, 1], [1, K]]))
            (act if h == 0 else nc.sync).dma_start(
                out=out_ap[:, o + K:o + 128], in_=out_t[:, o + K:o + 128])


_NC_CACHE = {}


def build():
    if "nc" in _NC_CACHE:
        return _NC_CACHE["nc"]
    nc = bacc.Bacc(
        "TRN2",
        target_bir_lowering=False,
        debug=False,
        enable_asserts=False,
        num_devices=N_CORES,
    )
    ca_d = nc.dram_tensor("cha", (P, CHA), DT, kind="ExternalInput")
    cb_d = nc.dram_tensor("chb", (P, CHB), DT, kind="ExternalInput")
    ps_d = nc.dram_tensor("pshift", (P, P), DT, kind="ExternalInput")
    g1_d = nc.dram_tensor("g1", (P, M * Y), DT, kind="ExternalInput")
    g2_d = nc.dram_tensor("g2", (P, M * Y), DT, kind="ExternalInput")
    g3_d = nc.dram_tensor("g3", (P, M * K), DT, kind="ExternalInput")
    out_d = nc.dram_tensor("out", (P, B * K), DT, kind="ExternalOutput")
    with TileContext(nc) as tc:
        _emit(tc, nc, ca_d.ap(), cb_d.ap(), ps_d.ap(), g1_d.ap(), g2_d.ap(),
              g3_d.ap(), out_d.ap())
    nc.compile()
    _NC_CACHE["nc"] = nc
    return nc


def _moebius_coeffs(toggle_gates):
    """sigmoid + Moebius transform of the gate maps -> multilinear coeffs.

    Input-independent weight preprocessing (exact math); returns
    (L, 16, d1, d2) float32 with m = bA*8 + bB*4 + bC*2 + bD.
    """
    tg = np.asarray(toggle_gates, dtype=np.float64)
    c = 1.0 / (1.0 + np.exp(-tg))                       # sigmoid
    c = c.reshape(L, 2, 2, 2, 2, P, Y)                  # (l, bA, bB, bC, bD, x, y)
    for ax in (1, 2, 3, 4):
        hi = [slice(None)] * 7
        lo = [slice(None)] * 7
        hi[ax] = 1
        lo[ax] = 0
        c[tuple(hi)] -= c[tuple(lo)]
    return c.reshape(L, M, P, Y).astype(np.float32)


def make_in_maps(x, toggle_gates):
    x = np.asarray(x, dtype=np.float32)
    c = _moebius_coeffs(toggle_gates)
    # layer 0: only S within {A,C} (even outputs) / {B,D} (odd) survive.
    # mult coeffs (s,h): [cA_e, cB_o, cAC_e, cBD_o]; add: [c0e, c0o, cCe, cDo]
    ev, od = c[0][:, :, 0::2], c[0][:, :, 1::2]
    g0m = np.stack([ev[8], od[4], ev[10], od[5]])
    g0a = np.stack([ev[0], od[0], ev[2], od[1]])
    g0m = g0m.transpose(1, 0, 2).reshape(P, 4 * K)
    g0a = g0a.transpose(1, 0, 2).reshape(P, 4 * K)

    def gl(l):
        a = c[l].transpose(1, 0, 2).reshape(P, M, K, 2)    # (P, m, k, t)
        return np.ascontiguousarray(
            a.transpose(0, 1, 3, 2).reshape(P, M * Y), dtype=np.float16)

    g1, g2 = gl(1), gl(2)
    g3 = np.ascontiguousarray(
        c[3][:, :, 0::2].transpose(1, 0, 2).reshape(P, M * K), dtype=np.float16)
    psm = np.eye(P, k=-1, dtype=np.float64)
    psm[0, P - 1] = 1.0
    psm = psm.astype(np.float16)
    xr = np.roll(x, -1, axis=1)                            # row shift (x+1)
    ins = []
    for cc in range(N_CORES):
        xs = x[cc * B:(cc + 1) * B]                        # (B, P, K)
        xf = xs.transpose(1, 0, 2).reshape(P, B, K)
        xrf = xr[cc * B:(cc + 1) * B].transpose(1, 0, 2).reshape(P, B, K)
        xcf = np.roll(xf, -1, axis=2)
        xrcf = np.roll(xrf, -1, axis=2)
        xp4 = np.stack([xf, xcf, xf, xcf], axis=2).reshape(P, B * 4 * K)
        xr2 = np.stack([xrf, xrcf], axis=2).reshape(P, B * 2 * K)
        cha = np.ascontiguousarray(
            np.concatenate([g0m, g0a, xp4], axis=1), dtype=np.float16)
        chb = np.ascontiguousarray(xr2, dtype=np.float16)
        ins.append({"cha": cha, "chb": chb, "pshift": psm,
                    "g1": g1, "g2": g2, "g3": g3})
    return ins


def kernel(x, toggle_gates):
    nc = build()
    res = run_bass_kernel_spmd(
        nc, make_in_maps(x, toggle_gates), core_ids=list(range(N_CORES))
    )
    outs = []
    for cc in range(N_CORES):
        o = res.results[cc]["out"].reshape(P, B, K).transpose(1, 0, 2)
        outs.append(o)
    return np.ascontiguousarray(np.concatenate(outs, axis=0), dtype=np.float32)


# revision 25
# speedup vs baseline: 1.0340x; 1.0100x over previous
"""Trainium2 Bass kernel for the soft-logic-gate CA problem.

Math (per sample, grid 128x128, 4 layers):
  state' = clip( sum_m sigmoid(tg[l,m]) * prod_j g(bit_j(m), tap_j), 0, 1 )
  taps: A=state[x,y], B=state[x,y+1], C=state[x+1,y], D=state[x+1,y+1] (periodic)
  g(0,t)=1-t, g(1,t)=t;  m = bA*8 + bB*4 + bC*2 + bD.

4-D multilinear interpolation of the 16 gate maps at corner (A,B,C,D).
The sigmoided gates are converted OFFLINE (host numpy, input-independent
weight preprocessing) to multilinear-polynomial coefficients via the
Moebius transform (c[m] -= c[m-bit]); the device evaluates each layer
with a Horner butterfly of fp16 tensor_tensor ops (A, then B, C, D).

Layout: partition = grid row (128).  State is parity planes (b, t, k):
t=0 even grid cols, t=1 odd.  The periodic column shift y+1 (B/D taps)
is materialized once per layer by the ACT engine into contiguous tap
tiles (stB from state, srB from the rowshift PSUM result), so every DVE
Horner op is a single large 2x-mode tensor_tensor with no wrap-column
splits.  Row shifts (x+1): layer 0 reads host-prepped rolled copies of
x; layers 1-3 use a PE permutation matmul + ACT copy-back.  All
coefficients arrive fp16 in final (m, t, k) layouts: no on-chip
sigmoid, Moebius, or casts.  Layer 0 ships only its 8 live coefficients
(B=D=0 initially), layer 3 only even-column ones.

Sharding: batch 32 -> 8 cores x 4 samples (coefficients replicated).
Engines: DVE does all Horner + clamps (GpSimd stays idle: its SBUF port
is shared with the DVE 2x read port, so concurrent Pool work stalls
DVE); ACT builds tap tiles + PSUM copy-backs; PE the row-shift matmuls;
DMA on the two HW-DGE queues (sync, scalar).
"""

import numpy as np

import concourse.bacc as bacc
import concourse.mybir as mybir
from concourse.ap import AP
from concourse.tile import TileContext
from concourse.bass_utils import run_bass_kernel_spmd

F32 = mybir.dt.float32
DT = mybir.dt.float16  # compute dtype
AL = mybir.AluOpType
P = 128          # partitions = grid rows
B = 4            # samples per core
Y = 128          # grid cols
K = 64           # x cols (even grid cols)
L = 4
M = 16
N_CORES = 8

SK = 2 * K       # state elems per sample (E|O planes)
CHA = 256 + B * 2 * K   # [g0 mult coeffs | XP2 (b: X Xc)]
CHB = 256 + B * 2 * K   # [g0 add coeffs | XR (b: Xr Xrc)]
O_L2, O_L3 = M * Y, 2 * M * Y
CW = 2 * M * Y + M * K


def _emit(tc, nc, ca_ap, cb_ap, ps_ap, g1_ap, g2_ap, g3_ap, out_ap):
    vec, act = nc.vector, nc.scalar

    def mk(t, off, dims):
        a = t if isinstance(t, AP) else t[:]
        return AP(a.tensor, a.offset + off, [list(a.ap[0])] + dims)

    def tt_(eng, out, in0, in1, op):
        eng.tensor_tensor(out=out, in0=in0, in1=in1, op=op)

    def clamp(out_ap_, in_ap_):
        vec.tensor_scalar(
            out=out_ap_, in0=in_ap_, scalar1=0.0, scalar2=1.0, op0=AL.max, op1=AL.min
        )

    with (
        tc.tile_pool(name="coef", bufs=1) as pc,
        tc.tile_pool(name="st", bufs=2) as pst,
        tc.tile_pool(name="sb", bufs=2) as psb,
        tc.tile_pool(name="sr", bufs=2) as psr,
        tc.tile_pool(name="wk", bufs=1) as pwk,
        tc.tile_pool(name="ps", bufs=2, space="PSUM") as pps,
    ):
        # ---- input DMAs, split across the two HW-DGE queues (FIFO per
        # queue: front-load what layer 0 needs).
        tw = pc.tile([P, CW], DT, tag="tw")
        cha = pwk.tile([P, CHA], DT, tag="cha")
        chb = pwk.tile([P, CHB], DT, tag="chb")
        nc.sync.dma_start(out=cha[:], in_=ca_ap)
        act.dma_start(out=chb[:], in_=cb_ap)
        # split g1 across both queues: the A-level mult needs the hi
        # (m=8..15) coefficient block first
        HB = 8 * Y
        nc.sync.dma_start(out=tw[:, HB:2 * HB], in_=g1_ap[:, HB:2 * HB])
        act.dma_start(out=tw[:, 0:HB], in_=g1_ap[:, 0:HB])
        psh = pwk.tile([P, P], DT, tag="psh")
        act.dma_start(out=psh[:], in_=ps_ap)
        nc.sync.dma_start(out=tw[:, O_L2:O_L2 + M * Y], in_=g2_ap)
        act.dma_start(out=tw[:, O_L3:O_L3 + M * K], in_=g3_ap)

        # warm the ACT table bank early so the first copy isn't stuck
        # behind a table load
        scr = pwk.tile([P, 2], F32, tag="scr")
        vec.memset(scr[:], 0.0)
        act.copy(out=scr[:, 1:2], in_=scr[:, 0:1])

        # ---- layer 0 eval: one fused 2-D interp over both parities ----
        # state layout (b, t, k): b*128 + t*64 + k
        # ue layout (b, s2, h2, k): both parity halves in one op set:
        #   h=0 even: s = (c0e + cA*X) + Xr*(cCe + cAC*X)   -> E plane
        #   h=1 odd:  s = (c0o + cB*Xc) + Xrc*(cDo + cBD*Xc) -> O plane
        # cha = [ [cA_e cB_o cAC_e cBD_o] | XP4 (b: X Xc X Xc) ]
        # chb = [ [c0_e c0_o cC_e cD_o]   | XR  (b: Xr Xrc) ]
        st1 = pst.tile([P, B * SK], DT, tag="state")
        ue = pwk.tile([P, 4 * B * K], DT, tag="ue")   # (b, s, h, k)
        te = pwk.tile([P, 2 * B * K], DT, tag="te")   # (b, h, k)

        for s in (0, 1):
            tt_(vec, mk(ue, s * 128, [[256, B], [64, 2], [1, K]]),
                mk(cha, s * 128, [[0, B], [64, 2], [1, K]]),
                mk(cha, 256, [[128, B], [64, 2], [1, K]]), AL.mult)
        tt_(vec, mk(ue, 0, [[256, B], [64, 4], [1, K]]),
            mk(ue, 0, [[256, B], [64, 4], [1, K]]),
            mk(chb, 0, [[0, B], [64, 4], [1, K]]), AL.add)
        tt_(vec, mk(te, 0, [[128, B], [64, 2], [1, K]]),
            mk(ue, 128, [[256, B], [64, 2], [1, K]]),
            mk(chb, 256, [[128, B], [64, 2], [1, K]]), AL.mult)
        tt_(vec, mk(te, 0, [[128, B], [64, 2], [1, K]]),
            mk(te, 0, [[128, B], [64, 2], [1, K]]),
            mk(ue, 0, [[256, B], [64, 2], [1, K]]), AL.add)
        clamp(mk(st1, 0, [[128, B], [1, 128]]), mk(te, 0, [[128, B], [1, 128]]))

        # ---- column-shift tap builder (ACT): dst(b,t,k) = y+1 taps ----
        def colshift(src, src_psum=False):
            dst = psb.tile([P, B * SK], DT, tag="stB")
            act.copy(out=mk(dst, 0, [[128, B], [1, K]]),
                     in_=mk(src, 64, [[128, B], [1, K]]))
            act.copy(out=mk(dst, 64, [[128, B], [1, K - 1]]),
                     in_=mk(src, 1, [[128, B], [1, K - 1]]))
            act.copy(out=mk(dst, 127, [[128, B], [1, 1]]),
                     in_=mk(src, 0, [[128, B], [1, 1]]))
            return dst

        # ---- generic layer eval (A, then B, C, D) ---------------------
        u = pwk.tile([P, 8 * B * SK], DT, tag="u")    # (b, i8, t, k)
        v_t = pwk.tile([P, 4 * B * SK], DT, tag="v")  # (b, j4, t, k)
        w2 = pwk.tile([P, 2 * B * SK], DT, tag="w2")  # (b, j2, t, k)
        tt2 = pwk.tile([P, B * SK], DT, tag="tt")     # (b, t, k)

        def eval_layer12(cofs, st, stB, sr, srB, stn):
            # A level: u_i = cLO_i + cHI_i * A
            tt_(vec, mk(u, 0, [[1024, B], [128, 8], [1, 128]]),
                mk(tw, cofs + 8 * Y, [[0, B], [128, 8], [1, 128]]),
                mk(st, 0, [[128, B], [0, 8], [1, 128]]), AL.mult)
            tt_(vec, mk(u, 0, [[1024, B], [128, 8], [1, 128]]),
                mk(u, 0, [[1024, B], [128, 8], [1, 128]]),
                mk(tw, cofs, [[0, B], [128, 8], [1, 128]]), AL.add)
            # B level: v_j = u_j + u_{4+j} * Btap
            tt_(vec, mk(v_t, 0, [[512, B], [128, 4], [1, 128]]),
                mk(u, 512, [[1024, B], [128, 4], [1, 128]]),
                mk(stB, 0, [[128, B], [0, 4], [1, 128]]), AL.mult)
            tt_(vec, mk(v_t, 0, [[512, B], [128, 4], [1, 128]]),
                mk(v_t, 0, [[512, B], [128, 4], [1, 128]]),
                mk(u, 0, [[1024, B], [128, 4], [1, 128]]), AL.add)
            # C level: w_j = v_j + v_{2+j} * C          (C = sr planes)
            tt_(vec, mk(w2, 0, [[256, B], [128, 2], [1, 128]]),
                mk(v_t, 256, [[512, B], [128, 2], [1, 128]]),
                mk(sr, 0, [[128, B], [0, 2], [1, 128]]), AL.mult)
            tt_(vec, mk(w2, 0, [[256, B], [128, 2], [1, 128]]),
                mk(w2, 0, [[256, B], [128, 2], [1, 128]]),
                mk(v_t, 0, [[512, B], [128, 2], [1, 128]]), AL.add)
            # D level: s = w_0 + w_1 * Dtap
            tt_(vec, mk(tt2, 0, [[128, B], [1, 128]]),
                mk(w2, 128, [[256, B], [1, 128]]),
                mk(srB, 0, [[128, B], [1, 128]]), AL.mult)
            tt_(vec, mk(tt2, 0, [[128, B], [1, 128]]),
                mk(tt2, 0, [[128, B], [1, 128]]),
                mk(w2, 0, [[256, B], [1, 128]]), AL.add)
            clamp(stn[:], tt2[:])

        st = st1
        for l in (1, 2):
            cofs = 0 if l == 1 else O_L2
            # PE rowshift matmul first (independent engine), then ACT tap
            # builds ordered by when DVE consumes them: stB (B level),
            # sr (C level), srB (D level)
            pt = pps.tile([P, B * SK], F32, tag="psum")
            nc.tensor.matmul(pt[:], psh[:], st[:], start=True, stop=True)
            stB = colshift(st)
            sr = psr.tile([P, B * SK], DT, tag="sr")
            act.copy(out=sr[:], in_=pt[:])
            srB = colshift(pt)
            stn = pst.tile([P, B * SK], DT, tag="state")
            eval_layer12(cofs, st, stB, sr, srB, stn)
            st = stn

        # ---- layer 3 (even outputs only, plane taps, no wraps) --------
        pt3 = pps.tile([P, B * SK], F32, tag="psum")
        nc.tensor.matmul(pt3[:], psh[:], st[:], start=True, stop=True)
        sr3 = psr.tile([P, B * SK], DT, tag="sr")
        act.copy(out=sr3[:], in_=pt3[:])
        tt_(vec, mk(u, 0, [[512, B], [64, 8], [1, K]]),
            mk(tw, O_L3 + 8 * K, [[0, B], [64, 8], [1, K]]),
            mk(st, 0, [[128, B], [0, 8], [1, K]]), AL.mult)
        tt_(vec, mk(u, 0, [[512, B], [64, 8], [1, K]]),
            mk(u, 0, [[512, B], [64, 8], [1, K]]),
            mk(tw, O_L3, [[0, B], [64, 8], [1, K]]), AL.add)
        tt_(vec, mk(v_t, 0, [[256, B], [64, 4], [1, K]]),
            mk(u, 256, [[512, B], [64, 4], [1, K]]),
            mk(st, 64, [[128, B], [0, 4], [1, K]]), AL.mult)
        tt_(vec, mk(v_t, 0, [[256, B], [64, 4], [1, K]]),
            mk(v_t, 0, [[256, B], [64, 4], [1, K]]),
            mk(u, 0, [[512, B], [64, 4], [1, K]]), AL.add)
        tt_(vec, mk(w2, 0, [[128, B], [64, 2], [1, K]]),
            mk(v_t, 128, [[256, B], [64, 2], [1, K]]),
            mk(sr3, 0, [[128, B], [0, 2], [1, K]]), AL.mult)
        tt_(vec, mk(w2, 0, [[128, B], [64, 2], [1, K]]),
            mk(w2, 0, [[128, B], [64, 2], [1, K]]),
            mk(v_t, 0, [[256, B], [64, 2], [1, K]]), AL.add)
        # D level + output: split by b-halves, alternate stores across
        # both HW-DGE queues so the tail store is never queued behind one
        out_t = pwk.tile([P, B * K], DT, tag="out")
        for h in (0, 1):
            o = h * 128          # tt2/out_t half offset (b-stride 64)
            q = h * 256          # w2/sr3 half offset (b-stride 128)
            tt_(vec, mk(tt2, o, [[64, 2], [1, K]]),
                mk(w2, 64 + q, [[128, 2], [1, K]]),
                mk(sr3, 64 + q, [[128, 2], [1, K]]), AL.mult)
            tt_(vec, mk(tt2, o, [[64, 2], [1, K]]),
                mk(tt2, o, [[64, 2], [1, K]]),
                mk(w2, q, [[128, 2], [1, K]]), AL.add)
            clamp(mk(out_t, o, [[64, 1], [1, K]]), mk(tt2, o, [[64, 1], [1, K]]))
            (nc.sync if h == 0 else act).dma_start(
                out=out_ap[:, o:o + K], in_=out_t[:, o:o + K])
            clamp(mk(out_t, o + K, [[64, 1], [1, K]]),
                  mk(tt2, o + K, [[64, 1], [1, K]]))
            (act if h == 0 else nc.sync).dma_start(
                out=out_ap[:, o + K:o + 128], in_=out_t[:, o + K:o + 128])


_NC_CACHE = {}


def build():
    if "nc" in _NC_CACHE:
        return _NC_CACHE["nc"]
    nc = bacc.Bacc(
        "TRN2",
        target_bir_lowering=False,
        debug=False,
        enable_asserts=False,
        num_devices=N_CORES,
    )
    ca_d = nc.dram_tensor("cha", (P, CHA), DT, kind="ExternalInput")
    cb_d = nc.dram_tensor("chb", (P, CHB), DT, kind="ExternalInput")
    ps_d = nc.dram_tensor("pshift", (P, P), DT, kind="ExternalInput")
    g1_d = nc.dram_tensor("g1", (P, M * Y), DT, kind="ExternalInput")
    g2_d = nc.dram_tensor("g2", (P, M * Y), DT, kind="ExternalInput")
    g3_d = nc.dram_tensor("g3", (P, M * K), DT, kind="ExternalInput")
    out_d = nc.dram_tensor("out", (P, B * K), DT, kind="ExternalOutput")
    with TileContext(nc) as tc:
        _emit(tc, nc, ca_d.ap(), cb_d.ap(), ps_d.ap(), g1_d.ap(), g2_d.ap(),
              g3_d.ap(), out_d.ap())
    nc.compile()
    _NC_CACHE["nc"] = nc
    return nc


def _moebius_coeffs(toggle_gates):
    """sigmoid + Moebius transform of the gate maps -> multilinear coeffs.

    Input-independent weight preprocessing (exact math); returns
    (L, 16, d1, d2) float32 with m = bA*8 + bB*4 + bC*2 + bD.
    """
    tg = np.asarray(toggle_gates, dtype=np.float64)
    c = 1.0 / (1.0 + np.exp(-tg))                       # sigmoid
    c = c.reshape(L, 2, 2, 2, 2, P, Y)                  # (l, bA, bB, bC, bD, x, y)
    for ax in (1, 2, 3, 4):
        hi = [slice(None)] * 7
        lo = [slice(None)] * 7
        hi[ax] = 1
        lo[ax] = 0
        c[tuple(hi)] -= c[tuple(lo)]
    return c.reshape(L, M, P, Y).astype(np.float32)


def make_in_maps(x, toggle_gates):
    x = np.asarray(x, dtype=np.float32)
    c = _moebius_coeffs(toggle_gates)
    # layer 0: only S within {A,C} (even outputs) / {B,D} (odd) survive.
    # mult coeffs (s,h): [cA_e, cB_o, cAC_e, cBD_o]; add: [c0e, c0o, cCe, cDo]
    ev, od = c[0][:, :, 0::2], c[0][:, :, 1::2]
    g0m = np.stack([ev[8], od[4], ev[10], od[5]])
    g0a = np.stack([ev[0], od[0], ev[2], od[1]])
    g0m = g0m.transpose(1, 0, 2).reshape(P, 4 * K)
    g0a = g0a.transpose(1, 0, 2).reshape(P, 4 * K)

    def gl(l):
        a = c[l].transpose(1, 0, 2).reshape(P, M, K, 2)    # (P, m, k, t)
        return np.ascontiguousarray(
            a.transpose(0, 1, 3, 2).reshape(P, M * Y), dtype=np.float16)

    g1, g2 = gl(1), gl(2)
    g3 = np.ascontiguousarray(
        c[3][:, :, 0::2].transpose(1, 0, 2).reshape(P, M * K), dtype=np.float16)
    psm = np.eye(P, k=-1, dtype=np.float64)
    psm[0, P - 1] = 1.0
    psm = psm.astype(np.float16)
    xr = np.roll(x, -1, axis=1)                            # row shift (x+1)
    ins = []
    for cc in range(N_CORES):
        xs = x[cc * B:(cc + 1) * B]                        # (B, P, K)
        xf = xs.transpose(1, 0, 2).reshape(P, B, K)
        xrf = xr[cc * B:(cc + 1) * B].transpose(1, 0, 2).reshape(P, B, K)
        xcf = np.roll(xf, -1, axis=2)
        xrcf = np.roll(xrf, -1, axis=2)
        xp2 = np.stack([xf, xcf], axis=2).reshape(P, B * 2 * K)
        xr2 = np.stack([xrf, xrcf], axis=2).reshape(P, B * 2 * K)
        cha = np.ascontiguousarray(
            np.concatenate([g0m, xp2], axis=1), dtype=np.float16)
        chb = np.ascontiguousarray(
            np.concatenate([g0a, xr2], axis=1), dtype=np.float16)
        ins.append({"cha": cha, "chb": chb, "pshift": psm,
                    "g1": g1, "g2": g2, "g3": g3})
    return ins


def kernel(x, toggle_gates):
    nc = build()
    res = run_bass_kernel_spmd(
        nc, make_in_maps(x, toggle_gates), core_ids=list(range(N_CORES))
    )
    outs = []
    for cc in range(N_CORES):
        o = res.results[cc]["out"].reshape(P, B, K).transpose(1, 0, 2)
        outs.append(o)
    return np.ascontiguousarray(np.concatenate(outs, axis=0), dtype=np.float32)


# revision 26
# speedup vs baseline: 1.0565x; 1.0218x over previous
"""Trainium2 Bass kernel for the soft-logic-gate CA problem.

Math (per sample, grid 128x128, 4 layers):
  state' = clip( sum_m sigmoid(tg[l,m]) * prod_j g(bit_j(m), tap_j), 0, 1 )
  taps: A=state[x,y], B=state[x,y+1], C=state[x+1,y], D=state[x+1,y+1] (periodic)
  g(0,t)=1-t, g(1,t)=t;  m = bA*8 + bB*4 + bC*2 + bD.

4-D multilinear interpolation of the 16 gate maps at corner (A,B,C,D).
The sigmoided gates are converted OFFLINE (host numpy, input-independent
weight preprocessing) to multilinear-polynomial coefficients via the
Moebius transform (c[m] -= c[m-bit]); the device evaluates each layer
with a Horner butterfly of fp16 tensor_tensor ops (A, then B, C, D).

Layout: partition = grid row (128).  State is parity planes (b, t, k):
t=0 even grid cols, t=1 odd.  The periodic column shift y+1 (B/D taps)
is materialized once per layer by the ACT engine into contiguous tap
tiles (stB from state, srB from the rowshift PSUM result), so every DVE
Horner op is a single large 2x-mode tensor_tensor with no wrap-column
splits.  Row shifts (x+1): layer 0 reads host-prepped rolled copies of
x; layers 1-3 use a PE permutation matmul + ACT copy-back.  All
coefficients arrive fp16 in final (m, t, k) layouts: no on-chip
sigmoid, Moebius, or casts.  Layer 0 ships only its 8 live coefficients
(B=D=0 initially), layer 3 only even-column ones.

Sharding: batch 32 -> 8 cores x 4 samples (coefficients replicated).
Engines: DVE does all Horner + clamps (GpSimd stays idle: its SBUF port
is shared with the DVE 2x read port, so concurrent Pool work stalls
DVE); ACT builds tap tiles + PSUM copy-backs; PE the row-shift matmuls;
DMA on the two HW-DGE queues (sync, scalar).
"""

import numpy as np

import concourse.bacc as bacc
import concourse.mybir as mybir
from concourse.ap import AP
from concourse.tile import TileContext
from concourse.bass_utils import run_bass_kernel_spmd

F32 = mybir.dt.float32
DT = mybir.dt.float16  # compute dtype
AL = mybir.AluOpType
P = 128          # partitions = grid rows
B = 4            # samples per core
Y = 128          # grid cols
K = 64           # x cols (even grid cols)
L = 4
M = 16
N_CORES = 8

SK = 2 * K       # state elems per sample (E|O planes)
CHA = 256 + B * 2 * K   # [g0 mult coeffs | XP2 (b: X Xc)]
CHB = 256 + B * 2 * K   # [g0 add coeffs | XR (b: Xr Xrc)]
O_L2, O_L3 = M * Y, 2 * M * Y
CW = 2 * M * Y + M * K


def _emit(tc, nc, ca_ap, cb_ap, ps_ap, g1_ap, g2_ap, g3_ap, out_ap):
    vec, act = nc.vector, nc.scalar

    def mk(t, off, dims):
        a = t if isinstance(t, AP) else t[:]
        return AP(a.tensor, a.offset + off, [list(a.ap[0])] + dims)

    def tt_(eng, out, in0, in1, op):
        eng.tensor_tensor(out=out, in0=in0, in1=in1, op=op)

    def clamp(out_ap_, in_ap_):
        vec.tensor_scalar(
            out=out_ap_, in0=in_ap_, scalar1=0.0, scalar2=1.0, op0=AL.max, op1=AL.min
        )

    with (
        tc.tile_pool(name="coef", bufs=1) as pc,
        tc.tile_pool(name="st", bufs=2) as pst,
        tc.tile_pool(name="sb", bufs=2) as psb,
        tc.tile_pool(name="sr", bufs=2) as psr,
        tc.tile_pool(name="wk", bufs=1) as pwk,
        tc.tile_pool(name="ps", bufs=2, space="PSUM") as pps,
    ):
        # ---- input DMAs, split across the two HW-DGE queues (FIFO per
        # queue: front-load what layer 0 needs).
        tw = pc.tile([P, CW], DT, tag="tw")
        cha = pwk.tile([P, CHA], DT, tag="cha")
        chb = pwk.tile([P, CHB], DT, tag="chb")
        nc.sync.dma_start(out=cha[:], in_=ca_ap)
        act.dma_start(out=chb[:], in_=cb_ap)
        # split g1 across both queues: the A-level mult needs the hi
        # (m=8..15) coefficient block first
        HB = 8 * Y
        nc.sync.dma_start(out=tw[:, HB:2 * HB], in_=g1_ap[:, HB:2 * HB])
        act.dma_start(out=tw[:, 0:HB], in_=g1_ap[:, 0:HB])
        psh = pwk.tile([P, P], DT, tag="psh")
        act.dma_start(out=psh[:], in_=ps_ap)
        nc.sync.dma_start(out=tw[:, O_L2:O_L2 + M * Y], in_=g2_ap)
        act.dma_start(out=tw[:, O_L3:O_L3 + M * K], in_=g3_ap)

        # warm the ACT table bank early so the first copy isn't stuck
        # behind a table load
        scr = pwk.tile([P, 2], F32, tag="scr")
        vec.memset(scr[:], 0.0)
        act.copy(out=scr[:, 1:2], in_=scr[:, 0:1])

        # ---- layer 0 eval: one fused 2-D interp over both parities ----
        # state layout (b, t, k): b*128 + t*64 + k
        # ue layout (b, s2, h2, k): both parity halves in one op set:
        #   h=0 even: s = (c0e + cA*X) + Xr*(cCe + cAC*X)   -> E plane
        #   h=1 odd:  s = (c0o + cB*Xc) + Xrc*(cDo + cBD*Xc) -> O plane
        # cha = [ [cA_e cB_o cAC_e cBD_o] | XP4 (b: X Xc X Xc) ]
        # chb = [ [c0_e c0_o cC_e cD_o]   | XR  (b: Xr Xrc) ]
        st1 = pst.tile([P, B * SK], DT, tag="state")
        ue = pwk.tile([P, 4 * B * K], DT, tag="ue")   # (b, s, h, k)
        te = pwk.tile([P, 2 * B * K], DT, tag="te")   # (b, h, k)

        for s in (0, 1):
            tt_(vec, mk(ue, s * 128, [[256, B], [64, 2], [1, K]]),
                mk(cha, s * 128, [[0, B], [64, 2], [1, K]]),
                mk(cha, 256, [[128, B], [64, 2], [1, K]]), AL.mult)
        tt_(vec, mk(ue, 0, [[256, B], [64, 4], [1, K]]),
            mk(ue, 0, [[256, B], [64, 4], [1, K]]),
            mk(chb, 0, [[0, B], [64, 4], [1, K]]), AL.add)
        tt_(vec, mk(te, 0, [[128, B], [64, 2], [1, K]]),
            mk(ue, 128, [[256, B], [64, 2], [1, K]]),
            mk(chb, 256, [[128, B], [64, 2], [1, K]]), AL.mult)
        tt_(vec, mk(te, 0, [[128, B], [64, 2], [1, K]]),
            mk(te, 0, [[128, B], [64, 2], [1, K]]),
            mk(ue, 0, [[256, B], [64, 2], [1, K]]), AL.add)
        clamp(mk(st1, 0, [[128, B], [1, 128]]), mk(te, 0, [[128, B], [1, 128]]))

        # ---- column-shift tap builder (ACT): dst(b,t,k) = y+1 taps ----
        def colshift(src, src_psum=False):
            dst = psb.tile([P, B * SK], DT, tag="stB")
            act.copy(out=mk(dst, 0, [[128, B], [1, K]]),
                     in_=mk(src, 64, [[128, B], [1, K]]))
            act.copy(out=mk(dst, 64, [[128, B], [1, K - 1]]),
                     in_=mk(src, 1, [[128, B], [1, K - 1]]))
            act.copy(out=mk(dst, 127, [[128, B], [1, 1]]),
                     in_=mk(src, 0, [[128, B], [1, 1]]))
            return dst

        # ---- generic layer eval (A, then B, C, D) ---------------------
        u = pwk.tile([P, 8 * B * SK], DT, tag="u")    # (b, i8, t, k)
        v_t = pwk.tile([P, 4 * B * SK], DT, tag="v")  # (b, j4, t, k)
        w2 = pwk.tile([P, 2 * B * SK], DT, tag="w2")  # (b, j2, t, k)
        tt2 = pwk.tile([P, B * SK], DT, tag="tt")     # (b, t, k)

        def eval_layer12(cofs, st, stB, sr, srB, stn):
            # A level: u_i = cLO_i + cHI_i * A
            tt_(vec, mk(u, 0, [[1024, B], [128, 8], [1, 128]]),
                mk(tw, cofs + 8 * Y, [[0, B], [128, 8], [1, 128]]),
                mk(st, 0, [[128, B], [0, 8], [1, 128]]), AL.mult)
            tt_(vec, mk(u, 0, [[1024, B], [128, 8], [1, 128]]),
                mk(u, 0, [[1024, B], [128, 8], [1, 128]]),
                mk(tw, cofs, [[0, B], [128, 8], [1, 128]]), AL.add)
            # B level: v_j = u_j + u_{4+j} * Btap
            tt_(vec, mk(v_t, 0, [[512, B], [128, 4], [1, 128]]),
                mk(u, 512, [[1024, B], [128, 4], [1, 128]]),
                mk(stB, 0, [[128, B], [0, 4], [1, 128]]), AL.mult)
            tt_(vec, mk(v_t, 0, [[512, B], [128, 4], [1, 128]]),
                mk(v_t, 0, [[512, B], [128, 4], [1, 128]]),
                mk(u, 0, [[1024, B], [128, 4], [1, 128]]), AL.add)
            # C level: w_j = v_j + v_{2+j} * C          (C = sr planes)
            tt_(vec, mk(w2, 0, [[256, B], [128, 2], [1, 128]]),
                mk(v_t, 256, [[512, B], [128, 2], [1, 128]]),
                mk(sr, 0, [[128, B], [0, 2], [1, 128]]), AL.mult)
            tt_(vec, mk(w2, 0, [[256, B], [128, 2], [1, 128]]),
                mk(w2, 0, [[256, B], [128, 2], [1, 128]]),
                mk(v_t, 0, [[512, B], [128, 2], [1, 128]]), AL.add)
            # D level: s = w_0 + w_1 * Dtap
            tt_(vec, mk(tt2, 0, [[128, B], [1, 128]]),
                mk(w2, 128, [[256, B], [1, 128]]),
                mk(srB, 0, [[128, B], [1, 128]]), AL.mult)
            tt_(vec, mk(tt2, 0, [[128, B], [1, 128]]),
                mk(tt2, 0, [[128, B], [1, 128]]),
                mk(w2, 0, [[256, B], [1, 128]]), AL.add)
            clamp(stn[:], tt2[:])

        st = st1
        for l in (1, 2):
            cofs = 0 if l == 1 else O_L2
            # PE rowshift matmul first (independent engine), then ACT tap
            # builds ordered by when DVE consumes them: stB (B level),
            # sr (C level), srB (D level)
            pt = pps.tile([P, B * SK], F32, tag="psum")
            nc.tensor.matmul(pt[:], psh[:], st[:], start=True, stop=True)
            stB = colshift(st)
            sr = psr.tile([P, B * SK], DT, tag="sr")
            act.copy(out=sr[:], in_=pt[:])
            srB = colshift(pt)
            stn = pst.tile([P, B * SK], DT, tag="state")
            eval_layer12(cofs, st, stB, sr, srB, stn)
            st = stn

        # ---- layer 3 (even outputs only, plane taps, no wraps) --------
        pt3 = pps.tile([P, B * SK], F32, tag="psum")
        nc.tensor.matmul(pt3[:], psh[:], st[:], start=True, stop=True)
        sr3 = psr.tile([P, B * SK], DT, tag="sr")
        act.copy(out=sr3[:], in_=pt3[:])
        tt_(vec, mk(u, 0, [[512, B], [64, 8], [1, K]]),
            mk(tw, O_L3 + 8 * K, [[0, B], [64, 8], [1, K]]),
            mk(st, 0, [[128, B], [0, 8], [1, K]]), AL.mult)
        tt_(vec, mk(u, 0, [[512, B], [64, 8], [1, K]]),
            mk(u, 0, [[512, B], [64, 8], [1, K]]),
            mk(tw, O_L3, [[0, B], [64, 8], [1, K]]), AL.add)
        tt_(vec, mk(v_t, 0, [[256, B], [64, 4], [1, K]]),
            mk(u, 256, [[512, B], [64, 4], [1, K]]),
            mk(st, 64, [[128, B], [0, 4], [1, K]]), AL.mult)
        tt_(vec, mk(v_t, 0, [[256, B], [64, 4], [1, K]]),
            mk(v_t, 0, [[256, B], [64, 4], [1, K]]),
            mk(u, 0, [[512, B], [64, 4], [1, K]]), AL.add)
        tt_(vec, mk(w2, 0, [[128, B], [64, 2], [1, K]]),
            mk(v_t, 128, [[256, B], [64, 2], [1, K]]),
            mk(sr3, 0, [[128, B], [0, 2], [1, K]]), AL.mult)
        tt_(vec, mk(w2, 0, [[128, B], [64, 2], [1, K]]),
            mk(w2, 0, [[128, B], [64, 2], [1, K]]),
            mk(v_t, 0, [[256, B], [64, 2], [1, K]]), AL.add)
        # D level + output: split by b-halves, alternate stores across
        # both HW-DGE queues so the tail store is never queued behind one
        out_t = pwk.tile([P, B * K], DT, tag="out")
        for h in (0, 1):
            o = h * 128          # tt2/out_t half offset (b-stride 64)
            q = h * 256          # w2/sr3 half offset (b-stride 128)
            tt_(vec, mk(tt2, o, [[64, 2], [1, K]]),
                mk(w2, 64 + q, [[128, 2], [1, K]]),
                mk(sr3, 64 + q, [[128, 2], [1, K]]), AL.mult)
            tt_(vec, mk(tt2, o, [[64, 2], [1, K]]),
                mk(tt2, o, [[64, 2], [1, K]]),
                mk(w2, q, [[128, 2], [1, K]]), AL.add)
            clamp(mk(out_t, o, [[64, 2], [1, K]]), mk(tt2, o, [[64, 2], [1, K]]))
            (nc.sync if h == 0 else act).dma_start(
                out=out_ap[:, o:o + 128], in_=out_t[:, o:o + 128])


_NC_CACHE = {}


def build():
    if "nc" in _NC_CACHE:
        return _NC_CACHE["nc"]
    nc = bacc.Bacc(
        "TRN2",
        target_bir_lowering=False,
        debug=False,
        enable_asserts=False,
        num_devices=N_CORES,
    )
    ca_d = nc.dram_tensor("cha", (P, CHA), DT, kind="ExternalInput")
    cb_d = nc.dram_tensor("chb", (P, CHB), DT, kind="ExternalInput")
    ps_d = nc.dram_tensor("pshift", (P, P), DT, kind="ExternalInput")
    g1_d = nc.dram_tensor("g1", (P, M * Y), DT, kind="ExternalInput")
    g2_d = nc.dram_tensor("g2", (P, M * Y), DT, kind="ExternalInput")
    g3_d = nc.dram_tensor("g3", (P, M * K), DT, kind="ExternalInput")
    out_d = nc.dram_tensor("out", (P, B * K), DT, kind="ExternalOutput")
    with TileContext(nc) as tc:
        _emit(tc, nc, ca_d.ap(), cb_d.ap(), ps_d.ap(), g1_d.ap(), g2_d.ap(),
              g3_d.ap(), out_d.ap())
    nc.compile()
    _NC_CACHE["nc"] = nc
    return nc


def _moebius_coeffs(toggle_gates):
    """sigmoid + Moebius transform of the gate maps -> multilinear coeffs.

    Input-independent weight preprocessing (exact math); returns
    (L, 16, d1, d2) float32 with m = bA*8 + bB*4 + bC*2 + bD.
    """
    tg = np.asarray(toggle_gates, dtype=np.float64)
    c = 1.0 / (1.0 + np.exp(-tg))                       # sigmoid
    c = c.reshape(L, 2, 2, 2, 2, P, Y)                  # (l, bA, bB, bC, bD, x, y)
    for ax in (1, 2, 3, 4):
        hi = [slice(None)] * 7
        lo = [slice(None)] * 7
        hi[ax] = 1
        lo[ax] = 0
        c[tuple(hi)] -= c[tuple(lo)]
    return c.reshape(L, M, P, Y).astype(np.float32)


def make_in_maps(x, toggle_gates):
    x = np.asarray(x, dtype=np.float32)
    c = _moebius_coeffs(toggle_gates)
    # layer 0: only S within {A,C} (even outputs) / {B,D} (odd) survive.
    # mult coeffs (s,h): [cA_e, cB_o, cAC_e, cBD_o]; add: [c0e, c0o, cCe, cDo]
    ev, od = c[0][:, :, 0::2], c[0][:, :, 1::2]
    g0m = np.stack([ev[8], od[4], ev[10], od[5]])
    g0a = np.stack([ev[0], od[0], ev[2], od[1]])
    g0m = g0m.transpose(1, 0, 2).reshape(P, 4 * K)
    g0a = g0a.transpose(1, 0, 2).reshape(P, 4 * K)

    def gl(l):
        a = c[l].transpose(1, 0, 2).reshape(P, M, K, 2)    # (P, m, k, t)
        return np.ascontiguousarray(
            a.transpose(0, 1, 3, 2).reshape(P, M * Y), dtype=np.float16)

    g1, g2 = gl(1), gl(2)
    g3 = np.ascontiguousarray(
        c[3][:, :, 0::2].transpose(1, 0, 2).reshape(P, M * K), dtype=np.float16)
    psm = np.eye(P, k=-1, dtype=np.float64)
    psm[0, P - 1] = 1.0
    psm = psm.astype(np.float16)
    xr = np.roll(x, -1, axis=1)                            # row shift (x+1)
    ins = []
    for cc in range(N_CORES):
        xs = x[cc * B:(cc + 1) * B]                        # (B, P, K)
        xf = xs.transpose(1, 0, 2).reshape(P, B, K)
        xrf = xr[cc * B:(cc + 1) * B].transpose(1, 0, 2).reshape(P, B, K)
        xcf = np.roll(xf, -1, axis=2)
        xrcf = np.roll(xrf, -1, axis=2)
        xp2 = np.stack([xf, xcf], axis=2).reshape(P, B * 2 * K)
        xr2 = np.stack([xrf, xrcf], axis=2).reshape(P, B * 2 * K)
        cha = np.ascontiguousarray(
            np.concatenate([g0m, xp2], axis=1), dtype=np.float16)
        chb = np.ascontiguousarray(
            np.concatenate([g0a, xr2], axis=1), dtype=np.float16)
        ins.append({"cha": cha, "chb": chb, "pshift": psm,
                    "g1": g1, "g2": g2, "g3": g3})
    return ins


def kernel(x, toggle_gates):
    nc = build()
    res = run_bass_kernel_spmd(
        nc, make_in_maps(x, toggle_gates), core_ids=list(range(N_CORES))
    )
    outs = []
    for cc in range(N_CORES):
        o = res.results[cc]["out"].reshape(P, B, K).transpose(1, 0, 2)
        outs.append(o)
    return np.ascontiguousarray(np.concatenate(outs, axis=0), dtype=np.float32)
